# revision 5
# baseline (speedup 1.0000x reference)
"""Class-attention with GFSA reaction term — Trainium2 Bass kernel, 8 NeuronCores.

Math (reference):
    q,k,v = x@W{q,k,v}.T split into H=12 heads of 64
    A  = softmax(q k^T / 8)                  per (b,h), [N,N], N=577
    attn = A + lamb_h * (3*A@A - 2*A)
    out  = (attn @ v) @ Wp.T + bp

Algebraic restructure (avoids the N^3 A@A entirely):
    out_head = (1-2l)*A@V + 3l*A@(A@V)
    With E = exp(logits) unnormalized and D = diag(rowsum(E)):
        U = E@[V|1] -> U, s   (ones-column trick, one fused matmul)
        W = (1-2l)*V + 3l*diag(1/s)*U
        out_head = diag(1/s) * (E @ W)

v2 (pipeline + engine-rebalance over v1):
  - per-batch V lives in ONE tile va_all[128, NT, H, 66] so the whole
    per-head DVE W-chain is 5 fused [128, NT, .] ops instead of 20.
  - U and Z accumulate in a 2-buffer ping-pong PSUM pool ("uz"), so
    U(h0) U(h1) Z(h0) Z(h1) never wait on each other's evictions.
  - pair transposes are deferred one pair and emitted between U and Z,
    filling the PE stall window while DVE computes W.
  - output is stored bf16 and upcast host-side (halves store traffic).

Sharding: pure data-parallel over batch, 2 batches per core, no collectives.
"""

import os
import sys

import numpy as np

for _p in ("/opt/trn_rl_repo", "/root/.axon_site", "/root/.axon_site/_ro/trn_rl_repo"):
    if _p not in sys.path and os.path.isdir(_p):
        sys.path.append(_p)

B, N, C, H, HD = 16, 577, 768, 12, 64
NCORES = 8
BPC = B // NCORES  # batches per core
SCALE = HD**-0.5
PAIRS = H // 2
CT = C // 128  # 6 c-tiles

TOK = [(t * 128, min(128, N - t * 128)) for t in range((N + 127) // 128)]
NT = len(TOK)  # 5
NP = N + 1  # 578 (even free dim; also the padded token count)
ETC = [(0, 512), (512, NP - 512)]  # logits free chunks (bank-aligned)
XQC = [(0, 512), (512, NP - 512)]  # q/k projection free chunks
PC = [(0, 384), (384, 384)]  # v / proj free chunks
HDP = HD + 2  # 66: V columns + ones column + pad

MODE = "bf16"

_CACHE = {}


def build(mode=MODE, loop_n=1):
    """Build + compile the per-core Bass program (SPMD, identical on all cores)."""
    import concourse.mybir as mybir
    import concourse.tile as tile
    from concourse import bacc
    from concourse.bass import broadcast_tensor_aps
    from concourse.masks import make_identity

    assert mode == "bf16", mode
    f32 = mybir.dt.float32
    bf16 = mybir.dt.bfloat16
    Exp = mybir.ActivationFunctionType.Exp
    mult, add = mybir.AluOpType.mult, mybir.AluOpType.add

    nc = bacc.Bacc("TRN2", target_bir_lowering=False, debug=False, num_devices=NCORES)

    xT = nc.dram_tensor("xT", [BPC, C, N], bf16, kind="ExternalInput")
    wqT = nc.dram_tensor("wqT", [C, C], bf16, kind="ExternalInput")
    wkT = nc.dram_tensor("wkT", [C, C], bf16, kind="ExternalInput")
    wvT = nc.dram_tensor("wvT", [C, C], bf16, kind="ExternalInput")
    wpT = nc.dram_tensor("wpT", [C, C], bf16, kind="ExternalInput")
    cst = nc.dram_tensor("cst", [128, 2, H], f32, kind="ExternalInput")
    onesH = nc.dram_tensor("onesH", [128, NT * H], bf16, kind="ExternalInput")
    out = nc.dram_tensor("out", [BPC, N, C], bf16, kind="ExternalOutput")

    with tile.TileContext(nc) as tc:
        with (
            tc.tile_pool(name="wp", bufs=1) as wpool,
            tc.tile_pool(name="cp", bufs=1) as cpool,
            tc.tile_pool(name="xp", bufs=2) as xpool,
            tc.tile_pool(name="qkp", bufs=2) as qkpool,
            tc.tile_pool(name="vap", bufs=2) as vapool,
            tc.tile_pool(name="etp", bufs=3) as etpool,
            tc.tile_pool(name="hsp", bufs=2) as hspool,
            tc.tile_pool(name="zcp", bufs=2) as zcpool,
            tc.tile_pool(name="obp", bufs=3) as obpool,
            tc.tile_pool(name="ps_et", bufs=2, space="PSUM") as ps_et,
            tc.tile_pool(name="ps_uz", bufs=2, space="PSUM") as ps_uz,
            tc.tile_pool(name="ps_io", bufs=2, space="PSUM") as ps_io,
        ):
            # ---- persistent constants / weights ----
            wq = [wpool.tile([128, C], bf16, tag=f"wq{ct}", name=f"wq{ct}") for ct in range(CT)]
            wk = [wpool.tile([128, C], bf16, tag=f"wk{ct}", name=f"wk{ct}") for ct in range(CT)]
            wv = [wpool.tile([128, C], bf16, tag=f"wv{ct}", name=f"wv{ct}") for ct in range(CT)]
            wp = [wpool.tile([128, C], bf16, tag=f"wp{ct}", name=f"wp{ct}") for ct in range(CT)]
            for ct in range(CT):
                sl = slice(ct * 128, (ct + 1) * 128)
                nc.scalar.dma_start(wq[ct][:], wqT[sl, :])
                nc.scalar.dma_start(wk[ct][:], wkT[sl, :])
            for ct in range(CT):
                sl = slice(ct * 128, (ct + 1) * 128)
                nc.scalar.dma_start(wv[ct][:], wvT[sl, :])
                nc.scalar.dma_start(wp[ct][:], wpT[sl, :])
            cst_sb = cpool.tile([128, 2, H], f32, tag="cst", name="cst_sb")
            nc.sync.dma_start(cst_sb[:], cst[:, :, :])
            ident = cpool.tile([128, 128], bf16, tag="id", name="ident")
            make_identity(nc, ident[:])

            def emit_pair_epilogue(zc, p, oh):
                """Transpose pair p's output block into zc rows (PE + DVE)."""
                for it, (t0, rows) in enumerate(TOK):
                    tps = ps_io.tile([128, 512], bf16, tag="io", name="tps")
                    nc.tensor.transpose(
                        tps[:, :rows], oh[:rows, it, :], ident[:rows, :rows]
                    )
                    nc.vector.tensor_copy(zc[p][:, t0 : t0 + rows], tps[:, :rows])

            def body():
                QT, KT, VA, ZC = {}, {}, {}, {}
                for b in range(BPC):
                    # ---- load x^T ----
                    xt = []
                    for ct in range(CT):
                        t_ = xpool.tile([128, NP], bf16, tag=f"xt{ct}", name=f"xt{ct}")
                        nc.sync.dma_start(t_[:, :N], xT[b, ct * 128 : (ct + 1) * 128, :])
                        xt.append(t_)
                    # ---- q^T, k^T  ([d,n] layout, head pair per 128-tile) ----
                    qt, kt = [], []
                    for name, w, dst in (("q", wq, qt), ("k", wk, kt)):
                        for dtt in range(CT):
                            o = qkpool.tile([128, NP], bf16, tag=f"{name}{dtt}", name=f"{name}t{dtt}")
                            for c0, cl in XQC:
                                ps = ps_io.tile([128, 512], f32, tag="io", name="iops")
                                for ct in range(CT):
                                    nc.tensor.matmul(
                                        ps[:, :cl],
                                        lhsT=w[ct][:, dtt * 128 : dtt * 128 + 128],
                                        rhs=xt[ct][:, c0 : c0 + cl],
                                        start=(ct == 0),
                                        stop=(ct == CT - 1),
                                    )
                                nc.vector.tensor_copy(o[:, c0 : c0 + cl], ps[:, :cl])
                            dst.append(o)
                    # ---- V (one tile [128, NT, H, 66]: V cols + ones col) ----
                    va = vapool.tile([128, NT, H, HDP], bf16, tag="va", name="va_all")
                    nc.sync.dma_start(
                        va[:, :, :, HD : HD + 1], onesH[:, :]
                    )
                    for ti, (t0, rows) in enumerate(TOK):
                        for half, (m0, ml) in enumerate(PC):
                            ps = ps_io.tile([128, 512], f32, tag="io", name="iops")
                            for ct in range(CT):
                                nc.tensor.matmul(
                                    ps[:rows, :ml],
                                    lhsT=xt[ct][:, t0 : t0 + rows],
                                    rhs=wv[ct][:, m0 : m0 + ml],
                                    start=(ct == 0),
                                    stop=(ct == CT - 1),
                                )
                            nc.scalar.copy(
                                va[:rows, ti, 6 * half : 6 * half + 6, :HD],
                                ps[:rows, :ml].rearrange("p (h d) -> p h d", d=HD),
                            )
                    QT[b], KT[b], VA[b] = qt, kt, va

                for b in range(BPC):
                    qt, kt, va = QT[b], KT[b], VA[b]
                    zc = [
                        zcpool.tile([128, N], bf16, tag=f"zc{ct}", name=f"zc{ct}")
                        for ct in range(CT)
                    ]
                    ZC[b] = zc
                    pending = None  # deferred (p, oh) transposes
                    for p in range(PAIRS):
                        # ---- E^T = exp(scale * K Q^T) per head of the pair ----
                        ets = []
                        for jt, (j0, jrows) in enumerate(TOK):
                            epair = []
                            for h01 in range(2):
                                lo = 64 * h01
                                eps = ps_et.tile([128, 1024], f32, tag="et", name="eps")
                                for i0, il in ETC:
                                    nc.tensor.matmul(
                                        eps[:jrows, i0 : i0 + il],
                                        lhsT=kt[p][lo : lo + 64, j0 : j0 + jrows],
                                        rhs=qt[p][lo : lo + 64, i0 : i0 + il],
                                        start=True,
                                        stop=True,
                                    )
                                esb = etpool.tile(
                                    [128, NP], bf16, tag=f"et{jt}_{h01}", name=f"esb{jt}_{h01}"
                                )
                                nc.scalar.activation(
                                    esb[:jrows, :], eps[:jrows, :NP], Exp, scale=SCALE
                                )
                                epair.append(esb)
                            ets.append(epair)

                        oh = hspool.tile([128, NT, 128], bf16, tag="oh", name="oh")
                        ups, srs, ws = {}, {}, {}
                        # U(h0), U(h1): independent accumulations, ping-pong psum
                        for h01 in range(2):
                            head = 2 * p + h01
                            up = ps_uz.tile([128, NT, HDP], f32, tag="uz", name="ups")
                            for jt, (j0, jrows) in enumerate(TOK):
                                for it, (t0, rows) in enumerate(TOK):
                                    nc.tensor.matmul(
                                        up[:rows, it, :HDP],
                                        lhsT=ets[jt][h01][:jrows, t0 : t0 + rows],
                                        rhs=va[:jrows, jt, head, :HDP],
                                        start=(jt == 0 and it == 0),
                                        stop=(jt == NT - 1),
                                    )
                            ups[h01] = up
                            # fused W-chain (DVE): sr5, csr, pp, w  [128, NT, .]
                            c1a = cst_sb[:, 0, head : head + 1]
                            c3a = cst_sb[:, 1, head : head + 1]
                            sr5 = hspool.tile([128, NT, 1], f32, tag=f"sr{h01}", name=f"sr{h01}")
                            nc.vector.reciprocal(sr5[:, :, :], up[:, :, HD : HD + 1])
                            csr = hspool.tile([128, NT, 1], f32, tag=f"csr{h01}", name=f"csr{h01}")
                            nc.vector.tensor_scalar_mul(csr[:, :, :], sr5[:, :, :], c3a)
                            pp = hspool.tile([128, NT, HD], bf16, tag=f"pp{h01}", name=f"pp{h01}")
                            i0b, i1b = broadcast_tensor_aps(up[:, :, :HD], csr[:, :, :])
                            nc.vector.tensor_tensor(pp[:, :, :], i0b, i1b, mult)
                            w_ = hspool.tile([128, NT, HD], bf16, tag=f"w{h01}", name=f"w{h01}")
                            nc.vector.scalar_tensor_tensor(
                                w_[:, :, :],
                                va[:, :, head, :HD],
                                c1a,
                                pp[:, :, :],
                                op0=mult,
                                op1=add,
                            )
                            srs[h01], ws[h01] = sr5, w_
                        # deferred transposes of the previous pair fill the
                        # PE window while DVE finishes W
                        if pending is not None:
                            emit_pair_epilogue(zc, *pending)
                        # Z(h0), Z(h1)
                        for h01 in range(2):
                            zp = ps_uz.tile([128, NT, HDP], f32, tag="uz", name="zps")
                            for jt, (j0, jrows) in enumerate(TOK):
                                for it, (t0, rows) in enumerate(TOK):
                                    nc.tensor.matmul(
                                        zp[:rows, it, :HD],
                                        lhsT=ets[jt][h01][:jrows, t0 : t0 + rows],
                                        rhs=ws[h01][:jrows, jt, :],
                                        start=(jt == 0 and it == 0),
                                        stop=(jt == NT - 1),
                                    )
                            # out_head = Z / s  (DVE, fused over all 5 tiles)
                            z0, s0 = broadcast_tensor_aps(
                                zp[:, :, :HD], srs[h01][:, :, :]
                            )
                            nc.vector.tensor_tensor(
                                oh[:, :, 64 * h01 : 64 * h01 + HD], z0, s0, mult
                            )
                        pending = (p, oh)
                    emit_pair_epilogue(zc, *pending)

                for b in range(BPC):
                    zc = ZC[b]
                    # ---- projection, then store (bias added host-side) ----
                    for it, (t0, rows) in enumerate(TOK):
                        ob = obpool.tile([128, C], bf16, tag="ob", name="ob")
                        for half, (m0, ml) in enumerate(PC):
                            pps = ps_io.tile([128, 512], f32, tag="io", name="iops")
                            for ct in range(CT):
                                nc.tensor.matmul(
                                    pps[:rows, :ml],
                                    lhsT=zc[ct][:, t0 : t0 + rows],
                                    rhs=wp[ct][:, m0 : m0 + ml],
                                    start=(ct == 0),
                                    stop=(ct == CT - 1),
                                )
                            nc.scalar.copy(ob[:rows, m0 : m0 + ml], pps[:rows, :ml])
                        nc.sync.dma_start(out[b, t0 : t0 + rows, :], ob[:rows, :])

            if loop_n > 1:
                with tc.For_i(0, loop_n, 1):
                    body()
            else:
                body()

    nc.compile()
    return nc


def _prep_in_maps(mode, x, Wq, Wk, Wv, Wp, bp, lamb):
    import ml_dtypes

    bf16 = ml_dtypes.bfloat16
    wqT = np.ascontiguousarray(Wq.T).astype(bf16)
    wkT = np.ascontiguousarray(Wk.T).astype(bf16)
    wvT = np.ascontiguousarray(Wv.T).astype(bf16)
    wpT = np.ascontiguousarray(Wp.T).astype(bf16)
    c1 = (1.0 - 2.0 * lamb).astype(np.float32)
    c3 = (3.0 * lamb).astype(np.float32)
    cstv = np.ascontiguousarray(
        np.broadcast_to(np.stack([c1, c3], 0)[None], (128, 2, H))
    ).astype(np.float32)
    onesHv = np.ones((128, NT * H), dtype=bf16)
    in_maps = []
    for core in range(NCORES):
        xs = x[core * BPC : (core + 1) * BPC]
        xTv = np.ascontiguousarray(xs.transpose(0, 2, 1)).astype(bf16)
        in_maps.append(
            dict(xT=xTv, wqT=wqT, wkT=wkT, wvT=wvT, wpT=wpT, cst=cstv, onesH=onesHv)
        )
    return in_maps


def kernel(x, Wq, Wk, Wv, Wp, bp, lamb):
    from concourse.bass_utils import run_bass_kernel_spmd

    x = np.asarray(x, dtype=np.float32)
    Wq = np.asarray(Wq, dtype=np.float32)
    Wk = np.asarray(Wk, dtype=np.float32)
    Wv = np.asarray(Wv, dtype=np.float32)
    Wp = np.asarray(Wp, dtype=np.float32)
    bp = np.asarray(bp, dtype=np.float32)
    lamb = np.asarray(lamb, dtype=np.float32)

    if MODE not in _CACHE:
        _CACHE[MODE] = build(MODE)
    nc = _CACHE[MODE]
    in_maps = _prep_in_maps(MODE, x, Wq, Wk, Wv, Wp, bp, lamb)
    res = run_bass_kernel_spmd(nc, in_maps, list(range(NCORES)))
    out = np.concatenate(
        [res.results[i]["out"].astype(np.float32) for i in range(NCORES)], axis=0
    )
    out += bp[None, None, :]
    return out
